# revision 61
# baseline (speedup 1.0000x reference)
"""Causal multi-head attention (8 heads, 1x1-conv projections) on 8 TRN2 cores.

Sharding: data-parallel over batch N=8 -> one batch element per NeuronCore.
Per-core kernel (S=1024 pixels, C=E=256 channels, H=8 heads, d=32):
  q = WqT.T @ x, k = WkT.T @ x              (e, s) layout, fp32r matmuls
  vT = x.T @ WvT                            (s, e) layout
  per head: P^T[sk, sq] = exp(mask(k_h^T q_h))   scores computed TRANSPOSED so
                                            softmax denominator comes from an
                                            appended ones-column in v (M=33)
  out_h = (vAug_h^T @ P^T) -> rows 0..31 numerator^T, row 32 = denominator
  att = num / denom (bf16), out = WprojT.T @ att + bproj_eff

Design (v2, ACT-stream-paced):
- The exp stream on the Activation engine is the pacing resource (~40us of
  exp work).  Everything else is scheduled around keeping it gap-free.
- Causal masking is folded into the score matmuls: an extra bf16 accumulate
  matmul (identity stationary x strictly-lower -1e30 moving) poisons the
  masked diagonal-block region of PSUM before the exp, so exp emits exact
  zeros and the DVE never touches masks.
- pts (the exp'd transposed scores) is PACKED per chunk (live columns
  only), which lets chunks 4+5 and 6+7 share one exp instruction each --
  saving the ~185ns fixed ACT cost per merged instruction.
- Per head h: attnv j0-half accumulates during head h (after exp chunk 3),
  the j1-half during head h+1 reusing the same [33,512] psum accumulator
  (ones-column in vaug gives the softmax denominator in row 32).
  Normalization: DVE reciprocal, gpsimd partition_broadcast, deferred DVE
  multiply into bf16 att.
- exp emission order per head: e1, e2, e3, e45, e0(next head), e67 --
  psum-ring write-after-read hazards then never block a score matmul
  near its exp slot.
- PSUM: 2x[128,1024] "bg" ring (chunks 0-3, 4+5, outproj pieces, warmup),
  1x[128,512] "c67" (chunks 6+7), 1x[33,512] "sm" (head-0 windows, tail),
  2x 2KB "pa" ring (attnv accumulators + q/k/v projection psum).
- Output projection + DMA run in 4 column pieces ([0:256], [256:512],
  [512:768], [768:1024]) pipelined against the last head's exp tail; the
  output bias enters as a bias-row x ones accumulate matmul, att and
  Wproj are bf16 so narrow matmuls run at full PE rate.
- Startup: x arrives as [0:256], [256:512], [512:1024] pieces; PE warmup
  matmuls start at ~700ns so the PE clock is ramped (3us rule) before the
  first projection; head 0 primes the stream with sub-window exps; m1
  projections and v-projection all run inside head 0's slack.
"""

import numpy as np

N_CORES = 8
C = 256      # input channels
E = 256      # embed channels (q/k)
O = 256      # v/out channels
S = 1024     # spatial positions (32*32)
H = 8        # heads
D = 32       # head dim
NCH = 2      # 256 = 2 * 128 partition chunks
N_WARM = 33  # PE clock-warm matmuls

_CACHE = {}


def _build_program():
    import concourse.mybir as mybir
    from concourse import bacc
    from concourse import library_config
    from concourse.tile import TileContext

    F32 = mybir.dt.float32
    F32R = mybir.dt.float32r
    BF16 = mybir.dt.bfloat16
    EXP = mybir.ActivationFunctionType.Exp

    nc = bacc.Bacc("TRN2", target_bir_lowering=False, debug=False)

    xin = nc.dram_tensor("xin", [C, S], F32R, kind="ExternalInput")
    wqk0 = nc.dram_tensor("wqk0", [C, 2 * 128], F32R, kind="ExternalInput")
    wqk1 = nc.dram_tensor("wqk1", [C, 2 * 128], F32R, kind="ExternalInput")
    wvt = nc.dram_tensor("wvt", [C, O], F32R, kind="ExternalInput")
    wpt = nc.dram_tensor("wpt", [O, O], BF16, kind="ExternalInput")
    biasd = nc.dram_tensor("biasd", [3 * 256], F32, kind="ExternalInput")
    bprowd = nc.dram_tensor("bprow", [1, 256], BF16, kind="ExternalInput")
    # msk2[:, 0:128] = identity, msk2[:, 128:256] = -1e30 * strict_lower
    msk2d = nc.dram_tensor("msk2", [128, 256], BF16, kind="ExternalInput")
    outd = nc.dram_tensor("out", [O, S], F32, kind="ExternalOutput")

    with TileContext(nc) as tc:
        with (
            tc.tile_pool(name="cst", bufs=1) as cst,
            tc.tile_pool(name="ptp", bufs=3) as ptp,
            tc.tile_pool(name="rbp", bufs=4) as rbp,
            tc.tile_pool(name="osb", bufs=4) as osb,
            tc.tile_pool(name="big", bufs=2, space="PSUM") as big,
            tc.tile_pool(name="sml", bufs=1, space="PSUM") as sml,
            tc.tile_pool(name="pap", bufs=2, space="PSUM") as pap,
        ):
            # --- fast start: Pool memsets, ACT table preload, PE clock warm
            wup = cst.tile([128, 64], F32, tag="wup")
            nc.gpsimd.memset(wup, 0.0)
            dmz = cst.tile([128, 1], F32, tag="dmz")
            nc.gpsimd.memset(dmz, 0.0)
            nc.gpsimd.load_library(library_config.attn)
            dme = cst.tile([128, 1], F32, tag="dme")
            nc.scalar.activation(dme, dmz, EXP)
            pwu = big.tile([64, 512], F32, tag="bg")
            for _ in range(N_WARM):
                nc.tensor.matmul(pwu[:, 0:64], wup.bitcast(F32R),
                                 wup[:, 0:64].bitcast(F32R),
                                 start=True, stop=True)

            vaug = cst.tile([128, 8, H, D + 1], BF16, tag="vaug")
            nc.vector.memset(vaug[:, :, :, D], 1.0)

            # --- input DMAs, ordered along the critical path
            wqk = cst.tile([128, NCH, NCH, 2, 128], F32R, tag="wqk")
            wqk_src = [
                d.ap().rearrange("(c p) (t e) -> p c t e", p=128, t=2)
                for d in (wqk0, wqk1)
            ]
            xr = cst.tile([128, NCH, S], F32R, tag="xr")
            xsrc = xin.ap().rearrange("(c p) s -> p c s", p=128)
            bt = cst.tile([128, 3, NCH], F32, tag="bt")
            msk2 = cst.tile([128, 256], BF16, tag="msk2")
            bprow = cst.tile([1, 256], BF16, tag="bprow")
            ones = cst.tile([1, 512], BF16, tag="ones")
            nc.vector.memset(ones, 1.0)
            wv = cst.tile([128, NCH, 256], F32R, tag="wv")
            wp = cst.tile([128, NCH, 256], BF16, tag="wp")

            nc.sync.dma_start(out=wqk[:, :, 0], in_=wqk_src[0])
            nc.sync.dma_start(out=xr[:, :, 0:256], in_=xsrc[:, :, 0:256])
            nc.sync.dma_start(
                out=bt, in_=biasd.ap().rearrange("(b m p) -> p b m", p=128, b=3)
            )
            nc.sync.dma_start(out=msk2, in_=msk2d.ap())
            nc.sync.dma_start(out=xr[:, :, 256:512], in_=xsrc[:, :, 256:512])
            nc.sync.dma_start(out=xr[:, :, 512:1024], in_=xsrc[:, :, 512:1024])
            nc.sync.dma_start(out=wqk[:, :, 1], in_=wqk_src[1])
            nc.sync.dma_start(out=wv, in_=wvt.ap().rearrange("(c p) e -> p c e", p=128))
            nc.sync.dma_start(out=wp, in_=wpt.ap().rearrange("(c p) e -> p c e", p=128))
            nc.sync.dma_start(out=bprow, in_=bprowd.ap())

            ident = msk2[:, 0:128]
            mlow = msk2[:, 128:256]

            q_sb = cst.tile([128, NCH, S], F32R, tag="q_sb")
            k_sb = cst.tile([128, NCH, S], F32R, tag="k_sb")
            att = cst.tile([128, NCH, S], BF16, tag="att")

            def qk_proj(t, m, c0, c1, eng):
                # t: 0 = q, 1 = k; columns [c0, c1); eng: 'dve' | 'act'
                # psum from the pa ring (same 2KB/partition slot size), which
                # is otherwise idle while projections run
                dst = (q_sb, k_sb)[t]
                pp = pap.tile([128, 512], F32, tag="pa")
                w = c1 - c0
                for c in range(2):
                    nc.tensor.matmul(
                        pp[:, 0:w],
                        wqk[:, c, m, t, :],
                        xr[:, c, c0:c1],
                        start=(c == 0), stop=(c == 1),
                    )
                if eng == 'act':
                    nc.scalar.add(dst[:, m, c0:c1], pp[:, 0:w], bt[:, t, m:m + 1])
                else:
                    nc.vector.tensor_scalar_add(
                        dst[:, m, c0:c1], pp[:, 0:w], bt[:, t, m:m + 1]
                    )

            # pts is PACKED: chunk i's live columns [128i, S) stored at
            # [POFF[i], POFF[i+1]), so merged exps write contiguous ranges
            POFF = [0, 1024, 1920, 2688, 3328, 3840, 4224, 4480, 4608]
            pts_tiles = {}

            def get_pts(h):
                if h not in pts_tiles:
                    pts = ptp.tile([128, POFF[8]], BF16, tag="pts",
                                   name=f"pts{h}")
                    pts_tiles[h] = pts
                return pts_tiles[h]

            def pcol(i, c):
                # pts column for chunk i, sq position c
                return POFF[i] + c - 128 * i

            def sc_win(h, i, ps, off, j):
                # score matmuls for sq window j of sk chunk i (+ PE mask on
                # the diagonal block window)
                m, r = h // 4, h % 4
                rows = slice(32 * r, 32 * r + 32)
                we = 512 * (j + 1)
                if we <= 128 * i:
                    return
                ws = max(512 * j, 128 * i)
                ws_mm = max(min(ws, we - 256), 512 * j)
                diag = ws == 128 * i  # window containing the diagonal block
                nc.tensor.matmul(
                    ps[:, ws_mm - off:we - off],
                    k_sb[rows, m, 128 * i:128 * (i + 1)],
                    q_sb[rows, m, ws_mm:we],
                    start=True, stop=not diag,
                    tile_position=(32 * r, 0),
                )
                if diag:
                    # psum[sk, sq] += -1e30 for sq < sk within the block
                    nc.tensor.matmul(
                        ps[:, 128 * i - off:128 * (i + 1) - off],
                        ident, mlow,
                        start=False, stop=True,
                    )

            def sc_exp(h, i, ps, off, e0, e1):
                nc.scalar.activation(
                    get_pts(h)[:, pcol(i, e0):pcol(i, e1)],
                    ps[:, e0 - off:e1 - off], EXP
                )

            def sc(h, i):
                # steady-state chunk 0-3: both windows + one exp (big ring)
                get_pts(h)
                ps = big.tile([128, S], F32, tag="bg", name=f"ps{h}_{i}")
                for j in range(2):
                    sc_win(h, i, ps, 0, j)
                sc_exp(h, i, ps, 0, 128 * i, S)

            def sc45(h):
                # chunks 4+5 share a big tile (c4 at [0:512], c5 at
                # [512:896]) and ONE exp over the packed pts range
                m, r = h // 4, h % 4
                rows = slice(32 * r, 32 * r + 32)
                ps = big.tile([128, S], F32, tag="bg", name=f"ps{h}_45")
                nc.tensor.matmul(ps[:, 0:512], k_sb[rows, m, 512:640],
                                 q_sb[rows, m, 512:1024], start=True,
                                 stop=False, tile_position=(32 * r, 0))
                nc.tensor.matmul(ps[:, 0:128], ident, mlow,
                                 start=False, stop=True)
                nc.tensor.matmul(ps[:, 512:896], k_sb[rows, m, 640:768],
                                 q_sb[rows, m, 640:1024], start=True,
                                 stop=False, tile_position=(32 * r, 0))
                nc.tensor.matmul(ps[:, 512:640], ident, mlow,
                                 start=False, stop=True)
                nc.scalar.activation(
                    get_pts(h)[:, POFF[4]:POFF[6]], ps[:, 0:896], EXP)

            def sc67(h):
                # chunks 6+7 share a small tile (c6 at [0:256], c7 at
                # [256:384], unwidened) and ONE exp
                m, r = h // 4, h % 4
                rows = slice(32 * r, 32 * r + 32)
                ps = sml.tile([128, 512], F32, tag="c67", name=f"ps{h}_67")
                nc.tensor.matmul(ps[:, 0:256], k_sb[rows, m, 768:896],
                                 q_sb[rows, m, 768:1024], start=True,
                                 stop=False, tile_position=(32 * r, 0))
                nc.tensor.matmul(ps[:, 0:128], ident, mlow,
                                 start=False, stop=True)
                nc.tensor.matmul(ps[:, 256:384], k_sb[rows, m, 896:1024],
                                 q_sb[rows, m, 896:1024], start=True,
                                 stop=False, tile_position=(32 * r, 0))
                nc.tensor.matmul(ps[:, 256:384], ident, mlow,
                                 start=False, stop=True)
                nc.scalar.activation(
                    get_pts(h)[:, POFF[6]:POFF[8]], ps[:, 0:384], EXP)

            def att_mms(h, pa, ii, q0, q1, first, last, base=0):
                # accumulate sq columns [q0, q1) into pa[:, q0-base:q1-base]
                pts = pts_tiles[h]
                for idx, i in enumerate(ii):
                    ws = max(q0, 128 * i)
                    nc.tensor.matmul(
                        pa[:, ws - base:q1 - base],
                        vaug[:, i, h, :],
                        pts[:, pcol(i, ws):pcol(i, q1)],
                        start=(first and idx == 0),
                        stop=(last and idx == len(ii) - 1),
                    )

            def att_recip_bcast(pa, p0, p1):
                w = p1 - p0
                rf = rbp.tile([1, 512], F32, tag="rf")
                nc.vector.reciprocal(rf[:, 0:w], pa[32:33, p0:p1])
                rb = rbp.tile([32, 512], F32, tag="rb")
                nc.gpsimd.partition_broadcast(rb[:, 0:w], rf[:, 0:w])
                return rb

            def att_mul(h, pa, rb, q0, q1, p0=0):
                # att columns [q0, q1) normalized from pa[0:32, p0:p0+w]
                m, r = h // 4, h % 4
                w = q1 - q0
                nc.vector.tensor_mul(
                    att[32 * r:32 * r + 32, m, q0:q1],
                    pa[0:32, p0:p0 + w], rb[:, 0:w],
                )

            out_ap = outd.ap().rearrange("(m p) s -> p m s", p=128)

            def po_piece(q0, q1, eng):
                # output projection for columns [q0, q1); the bias is folded
                # in as a bias-row x ones accumulate matmul, so the psum ->
                # SBUF move is a single plain copy per piece
                w = q1 - q0
                po = big.tile([128, 2, 512], F32, tag="bg", name=f"po{q0}")
                for m in range(2):
                    for c in range(2):
                        nc.tensor.matmul(
                            po[:, m, 0:w],
                            wp[:, c, m * 128:(m + 1) * 128],
                            att[:, c, q0:q1],
                            start=(c == 0), stop=False,
                        )
                    nc.tensor.matmul(
                        po[:, m, 0:w],
                        bprow[:, m * 128:(m + 1) * 128],
                        ones[:, 0:w],
                        start=False, stop=True,
                    )
                ot = osb.tile([128, 2, 512], F32, tag="ot", name=f"ot{q0}")
                if eng == 'act':
                    nc.scalar.copy(ot[:, :, 0:w], po[:, :, 0:w])
                else:
                    nc.vector.tensor_copy(ot[:, :, 0:w], po[:, :, 0:w])
                nc.sync.dma_start(out=out_ap[:, :, q0:q1], in_=ot[:, :, 0:w])

            # ---------------- head 0 priming ----------------
            def sc_win0(i, ps, off, w0, w1):
                # head-0 score mms + exp for sq window [w0, w1) of chunk i
                ws = max(w0, 128 * i)
                if w1 <= ws:
                    return
                ws_mm = max(min(ws, w1 - 256), w0)
                diag = ws == 128 * i
                nc.tensor.matmul(
                    ps[:, ws_mm - off:w1 - off],
                    k_sb[0:32, 0, 128 * i:128 * (i + 1)],
                    q_sb[0:32, 0, ws_mm:w1],
                    start=True, stop=not diag,
                    tile_position=(0, 0),
                )
                if diag:
                    nc.tensor.matmul(
                        ps[:, 128 * i - off:128 * (i + 1) - off],
                        ident, mlow,
                        start=False, stop=True,
                    )
                nc.scalar.activation(
                    get_pts(0)[:, pcol(i, ws):pcol(i, w1)],
                    ps[:, ws - off:w1 - off], EXP
                )

            qk_proj(0, 0, 0, 256, 'dve')
            qk_proj(1, 0, 0, 256, 'act')
            b0 = big.tile([128, S], F32, tag="bg", name="ps0_0")
            b1 = big.tile([128, S], F32, tag="bg", name="ps0_1")
            sc_win0(0, b0, 0, 0, 256)
            sc_win0(1, b1, 0, 0, 256)
            qk_proj(0, 0, 256, 512, 'dve')
            qk_proj(1, 0, 256, 512, 'act')
            sc_win0(0, b0, 0, 256, 512)
            sc_win0(1, b1, 0, 256, 512)
            # chunks 2/3's j0 windows share one small tile (disjoint cols)
            s23a = sml.tile([128, 512], F32, tag="sm", name="ps0_23a")
            sc_win0(2, s23a, 256, 256, 512)
            sc_win0(3, s23a, 0, 256, 512)
            # j1 windows (x second half); chunks 2/3 share one big tile
            qk_proj(0, 0, 512, 1024, 'dve')
            qk_proj(1, 0, 512, 1024, 'act')
            sc_win0(0, b0, 0, 512, 1024)
            sc_win0(1, b1, 0, 512, 1024)
            s23b = big.tile([128, S], F32, tag="bg", name="ps0_23b")
            sc_win0(2, s23b, 512, 512, 1024)
            sc_win0(3, s23b, 0, 512, 1024)
            sc45(0)
            sc(1, 0)
            sc67(0)

            # v projection (after wv lands) + head-0 attnv j0
            def v_proj(i):
                pv = pap.tile([128, 512], F32, tag="pa", name=f"pv{i}")
                for c in range(2):
                    nc.tensor.matmul(
                        pv[:, 0:256],
                        xr[:, c, i * 128:(i + 1) * 128],
                        wv[:, c, :],
                        start=(c == 0), stop=(c == 1),
                    )
                nc.vector.tensor_copy(
                    vaug[:, i, :, 0:D],
                    pv[:, 0:256].rearrange("p (h d) -> p h d", h=H),
                )

            # m1 q/k projections: wqk1 lands mid-head-0; doing these here
            # keeps their psum slots and bias-adds off the steady-state path
            qk_proj(0, 1, 0, 512, 'dve')
            qk_proj(1, 1, 0, 512, 'dve')
            qk_proj(0, 1, 512, 1024, 'dve')
            qk_proj(1, 1, 512, 1024, 'dve')

            for i in range(8):
                v_proj(i)

            # One [33,512] accumulator per head: j0 round in head h, then the
            # j1 round REUSES the same tile in h+1 (after the j0 multiply)
            pa_att = {}  # h -> accumulator tile
            pa_j0 = {}   # h -> rb for the j0 half

            def attn_j0(h):
                pa = pap.tile([33, 512], F32, tag="pa", name=f"pa{h}")
                pa_att[h] = pa
                att_mms(h, pa, [0, 1, 2, 3], 0, 512, True, True)
                pa_j0[h] = att_recip_bcast(pa, 0, 512)

            attn_j0(0)
            att_mul(0, pa_att[0], pa_j0.pop(0), 0, 512)

            # ---------------- steady heads ----------------
            def emit_steady(h):
                prev = h - 1
                for i in (1, 2, 3):
                    sc(h, i)
                sc45(h)
                # attnv j0 of h: mms ready after e3; PE reaches them here
                attn_j0(h)
                sc(h + 1, 0)
                sc67(h)
                # j0 multiply of h (dep: bcast just emitted)
                att_mul(h, pa_att[h], pa_j0.pop(h), 0, 512)
                # attnv j1 of prev, reusing its accumulator (j0 mul done)
                pa_prev = pa_att.pop(prev)
                att_mms(prev, pa_prev, [0, 1, 2, 3], 512, 1024, True, False,
                        base=512)
                att_mms(prev, pa_prev, [4, 5, 6, 7], 512, 1024, False, True,
                        base=512)
                rbj1 = att_recip_bcast(pa_prev, 0, 512)
                att_mul(prev, pa_prev, rbj1, 512, 1024, p0=0)
                pts_tiles.pop(prev)

            for h in range(1, 7):
                emit_steady(h)

            # ---------------- head 7 + tail ----------------
            p7 = get_pts(7)
            pa6 = pa_att.pop(6)
            sc(7, 1)
            sc(7, 2)
            # j1 of head 6, interleaved with head-7 scores
            att_mms(6, pa6, [0, 1], 512, 1024, True, False, base=512)
            sc(7, 3)
            att_mms(6, pa6, [2, 3], 512, 1024, False, False, base=512)
            sc45(7)
            # attnv j0 of head 7, group A: cols [0:256) needs chunks 0,1
            pa7 = pap.tile([33, 512], F32, tag="pa", name="pa7")
            nc.tensor.matmul(pa7[:, 0:256], vaug[:, 0, 7, :], p7[:, 0:256],
                             start=True, stop=False)
            nc.tensor.matmul(pa7[:, 128:256], vaug[:, 1, 7, :],
                             p7[:, pcol(1, 128):pcol(1, 256)],
                             start=False, stop=True)
            rb_a = att_recip_bcast(pa7, 0, 256)
            att_mms(6, pa6, [4, 5, 6, 7], 512, 1024, False, True, base=512)
            rbj1_6 = att_recip_bcast(pa6, 0, 512)
            # group B: cols [256:512) needs chunks 0-3
            nc.tensor.matmul(pa7[:, 256:512], vaug[:, 0, 7, :], p7[:, 256:512],
                             start=True, stop=False)
            nc.tensor.matmul(pa7[:, 256:512], vaug[:, 1, 7, :],
                             p7[:, pcol(1, 256):pcol(1, 512)],
                             start=False, stop=False)
            nc.tensor.matmul(pa7[:, 256:512], vaug[:, 2, 7, :],
                             p7[:, pcol(2, 256):pcol(2, 512)],
                             start=False, stop=False)
            nc.tensor.matmul(pa7[:, 384:512], vaug[:, 3, 7, :],
                             p7[:, pcol(3, 384):pcol(3, 512)],
                             start=False, stop=True)
            rb_b = att_recip_bcast(pa7, 256, 512)
            sc67(7)
            pts_tiles.pop(6)
            # tail accumulators: [512:768) at cols [0:256) and [768:1024)
            # at cols [256:512) of one shared small-ring tile
            pa_cde = sml.tile([33, 512], F32, tag="sm", name="pa_cde")
            att_mms(7, pa_cde, [0, 1, 2, 3, 4, 5], 512, 768, True, True,
                    base=512)
            att_mms(7, pa_cde, [0, 1, 2, 3, 4, 5], 768, 1024, True, False,
                    base=512)
            # muls + outproj pieces, pipelined against the exp tail
            att_mul(7, pa7, rb_a, 0, 256)
            po_piece(0, 256, 'act')
            rb_c = att_recip_bcast(pa_cde, 0, 256)
            att_mul(6, pa6, rbj1_6, 512, 1024, p0=0)
            att_mul(7, pa7, rb_b, 256, 512, p0=256)
            po_piece(256, 512, 'act')
            att_mms(7, pa_cde, [6, 7], 768, 1024, False, True, base=512)
            rb_de = att_recip_bcast(pa_cde, 256, 512)
            att_mul(7, pa_cde, rb_c, 512, 768, p0=0)
            po_piece(512, 768, 'act')
            att_mul(7, pa_cde, rb_de, 768, 1024, p0=256)
            po_piece(768, 1024, 'act')

    nc.compile()
    return nc


def get_program():
    if "nc" not in _CACHE:
        _CACHE["nc"] = _build_program()
    return _CACHE["nc"]


def kernel(x, wq, bq, wkv, bkv, wproj, bproj):
    import ml_dtypes
    from concourse.bass_utils import run_bass_kernel_spmd

    nc = get_program()

    x = np.asarray(x, dtype=np.float32)
    n = x.shape[0]
    assert n == N_CORES and x.shape[1:] == (C, 32, 32)

    scale = 1.0 / np.sqrt(np.float32(D))
    wq_s = np.asarray(wq, np.float32) * scale
    bq_s = np.asarray(bq, np.float32) * scale
    wk = np.asarray(wkv[:E], np.float32)
    bk = np.asarray(bkv[:E], np.float32)
    wv = np.asarray(wkv[E:], np.float32)
    bv = np.asarray(bkv[E:], np.float32)
    wproj = np.asarray(wproj, np.float32)
    bproj_eff = (np.asarray(bproj, np.float32)
                 + wproj.astype(np.float64) @ bv.astype(np.float64)).astype(np.float32)

    # msk2: [identity | -1e30 * strict_lower(r > sq)]
    ident = np.eye(128, dtype=np.float32)
    mlow = np.where(np.arange(128)[:, None] > np.arange(128)[None, :],
                    np.float32(-1e30), np.float32(0.0))
    msk2 = np.concatenate([ident, mlow], axis=1).astype(ml_dtypes.bfloat16)

    shared = {
        "wqk0": np.ascontiguousarray(
            np.concatenate([wq_s.T[:, 0:128], wk.T[:, 0:128]], axis=1)),
        "wqk1": np.ascontiguousarray(
            np.concatenate([wq_s.T[:, 128:256], wk.T[:, 128:256]], axis=1)),
        "wvt": np.ascontiguousarray(wv.T),
        "wpt": np.ascontiguousarray(wproj.T.astype(ml_dtypes.bfloat16)),
        "biasd": np.ascontiguousarray(
            np.concatenate([bq_s, bk, bproj_eff])),
        "bprow": np.ascontiguousarray(
            bproj_eff.reshape(1, 256).astype(ml_dtypes.bfloat16)),
        "msk2": np.ascontiguousarray(msk2),
    }
    in_maps = [
        {"xin": np.ascontiguousarray(x[i].reshape(C, S)), **shared}
        for i in range(N_CORES)
    ]
    res = run_bass_kernel_spmd(nc, in_maps, core_ids=list(range(N_CORES)))
    out = np.stack([res.results[i]["out"].reshape(O, 32, 32) for i in range(N_CORES)])
    return out.astype(np.float32)


# revision 66
# speedup vs baseline: 1.0016x; 1.0016x over previous
"""Causal multi-head attention (8 heads, 1x1-conv projections) on 8 TRN2 cores.

Sharding: data-parallel over batch N=8 -> one batch element per NeuronCore.
Per-core kernel (S=1024 pixels, C=E=256 channels, H=8 heads, d=32):
  q = WqT.T @ x, k = WkT.T @ x              (e, s) layout, fp32r matmuls
  vT = x.T @ WvT                            (s, e) layout
  per head: P^T[sk, sq] = exp(mask(k_h^T q_h))   scores computed TRANSPOSED so
                                            softmax denominator comes from an
                                            appended ones-column in v (M=33)
  out_h = (vAug_h^T @ P^T) -> rows 0..31 numerator^T, row 32 = denominator
  att = num / denom (bf16), out = WprojT.T @ att + bproj_eff

Design (v2, ACT-stream-paced):
- The exp stream on the Activation engine is the pacing resource (~40us of
  exp work).  Everything else is scheduled around keeping it gap-free.
- Causal masking is folded into the score matmuls: an extra bf16 accumulate
  matmul (identity stationary x strictly-lower -1e30 moving) poisons the
  masked diagonal-block region of PSUM before the exp, so exp emits exact
  zeros and the DVE never touches masks.
- pts (the exp'd transposed scores) is PACKED per chunk (live columns
  only), which lets chunks 4+5 and 6+7 share one exp instruction each --
  saving the ~185ns fixed ACT cost per merged instruction.
- Per head h: attnv j0-half accumulates during head h (after exp chunk 3),
  the j1-half during head h+1 reusing the same [33,512] psum accumulator
  (ones-column in vaug gives the softmax denominator in row 32).
  Normalization: DVE reciprocal, gpsimd partition_broadcast, deferred DVE
  multiply into bf16 att.
- exp emission order per head: e1, e2, e3, e45, e0(next head), e67 --
  psum-ring write-after-read hazards then never block a score matmul
  near its exp slot.
- PSUM: 2x[128,1024] "bg" ring (chunks 0-3, 4+5, outproj pieces, warmup),
  1x[128,512] "c67" (chunks 6+7), 1x[33,512] "sm" (head-0 windows, tail),
  2x 2KB "pa" ring (attnv accumulators + q/k/v projection psum).
- Output projection + DMA run in 4 column pieces ([0:256], [256:512],
  [512:768], [768:1024]) pipelined against the last head's exp tail; the
  output bias enters as a bias-row x ones accumulate matmul, att and
  Wproj are bf16 so narrow matmuls run at full PE rate.
- Startup: x arrives as [0:256], [256:512], [512:1024] pieces; PE warmup
  matmuls start at ~700ns so the PE clock is ramped (3us rule) before the
  first projection; head 0 primes the stream with sub-window exps; m1
  projections and v-projection all run inside head 0's slack.
"""

import numpy as np

N_CORES = 8
C = 256      # input channels
E = 256      # embed channels (q/k)
O = 256      # v/out channels
S = 1024     # spatial positions (32*32)
H = 8        # heads
D = 32       # head dim
NCH = 2      # 256 = 2 * 128 partition chunks
N_WARM = 33  # PE clock-warm matmuls

_CACHE = {}


def _build_program():
    import concourse.mybir as mybir
    from concourse import bacc
    from concourse import library_config
    from concourse.tile import TileContext

    F32 = mybir.dt.float32
    F32R = mybir.dt.float32r
    BF16 = mybir.dt.bfloat16
    EXP = mybir.ActivationFunctionType.Exp

    nc = bacc.Bacc("TRN2", target_bir_lowering=False, debug=False)

    xin = nc.dram_tensor("xin", [C, S], F32R, kind="ExternalInput")
    wqk0 = nc.dram_tensor("wqk0", [C, 2 * 128], F32R, kind="ExternalInput")
    wqk1 = nc.dram_tensor("wqk1", [C, 2 * 128], F32R, kind="ExternalInput")
    wvt = nc.dram_tensor("wvt", [C, O], F32R, kind="ExternalInput")
    wpt = nc.dram_tensor("wpt", [O, O], BF16, kind="ExternalInput")
    biasd = nc.dram_tensor("biasd", [3 * 256], F32, kind="ExternalInput")
    bprowd = nc.dram_tensor("bprow", [1, 256], BF16, kind="ExternalInput")
    # msk2[:, 0:128] = identity, msk2[:, 128:256] = -1e30 * strict_lower
    msk2d = nc.dram_tensor("msk2", [128, 256], BF16, kind="ExternalInput")
    outd = nc.dram_tensor("out", [O, S], F32, kind="ExternalOutput")

    with TileContext(nc) as tc:
        with (
            tc.tile_pool(name="cst", bufs=1) as cst,
            tc.tile_pool(name="ptp", bufs=3) as ptp,
            tc.tile_pool(name="rbp", bufs=4) as rbp,
            tc.tile_pool(name="osb", bufs=4) as osb,
            tc.tile_pool(name="big", bufs=2, space="PSUM") as big,
            tc.tile_pool(name="sml", bufs=1, space="PSUM") as sml,
            tc.tile_pool(name="pap", bufs=2, space="PSUM") as pap,
        ):
            # --- fast start: Pool memsets, ACT table preload, PE clock warm
            wup = cst.tile([128, 64], F32, tag="wup")
            nc.gpsimd.memset(wup, 0.0)
            dmz = cst.tile([128, 1], F32, tag="dmz")
            nc.gpsimd.memset(dmz, 0.0)
            nc.gpsimd.load_library(library_config.attn)
            dme = cst.tile([128, 1], F32, tag="dme")
            nc.scalar.activation(dme, dmz, EXP)
            pwu = big.tile([64, 512], F32, tag="bg")
            for _ in range(N_WARM):
                nc.tensor.matmul(pwu[:, 0:64], wup.bitcast(F32R),
                                 wup[:, 0:64].bitcast(F32R),
                                 start=True, stop=True)

            vaug = cst.tile([128, 8, H, D + 1], BF16, tag="vaug")
            nc.vector.memset(vaug[:, :, :, D], 1.0)

            # --- input DMAs, ordered along the critical path
            wqk = cst.tile([128, NCH, NCH, 2, 128], F32R, tag="wqk")
            wqk_src = [
                d.ap().rearrange("(c p) (t e) -> p c t e", p=128, t=2)
                for d in (wqk0, wqk1)
            ]
            xr = cst.tile([128, NCH, S], F32R, tag="xr")
            xsrc = xin.ap().rearrange("(c p) s -> p c s", p=128)
            bt = cst.tile([128, 3, NCH], F32, tag="bt")
            msk2 = cst.tile([128, 256], BF16, tag="msk2")
            bprow = cst.tile([1, 256], BF16, tag="bprow")
            ones = cst.tile([1, 512], BF16, tag="ones")
            nc.vector.memset(ones, 1.0)
            wv = cst.tile([128, NCH, 256], F32R, tag="wv")
            wp = cst.tile([128, NCH, 256], BF16, tag="wp")

            nc.sync.dma_start(out=wqk[:, :, 0], in_=wqk_src[0])
            nc.sync.dma_start(out=xr[:, :, 0:256], in_=xsrc[:, :, 0:256])
            nc.sync.dma_start(
                out=bt, in_=biasd.ap().rearrange("(b m p) -> p b m", p=128, b=3)
            )
            nc.sync.dma_start(out=msk2, in_=msk2d.ap())
            nc.sync.dma_start(out=xr[:, :, 256:512], in_=xsrc[:, :, 256:512])
            nc.sync.dma_start(out=xr[:, :, 512:1024], in_=xsrc[:, :, 512:1024])
            nc.sync.dma_start(out=wqk[:, :, 1], in_=wqk_src[1])
            nc.sync.dma_start(out=wv, in_=wvt.ap().rearrange("(c p) e -> p c e", p=128))
            nc.sync.dma_start(out=wp, in_=wpt.ap().rearrange("(c p) e -> p c e", p=128))
            nc.sync.dma_start(out=bprow, in_=bprowd.ap())

            ident = msk2[:, 0:128]
            mlow = msk2[:, 128:256]

            q_sb = cst.tile([128, NCH, S], F32R, tag="q_sb")
            k_sb = cst.tile([128, NCH, S], F32R, tag="k_sb")
            att = cst.tile([128, NCH, S], BF16, tag="att")

            def qk_proj(t, m, c0, c1, eng):
                # t: 0 = q, 1 = k; columns [c0, c1); eng: 'dve' | 'act'
                # psum from the pa ring (same 2KB/partition slot size), which
                # is otherwise idle while projections run
                dst = (q_sb, k_sb)[t]
                pp = pap.tile([128, 512], F32, tag="pa")
                w = c1 - c0
                for c in range(2):
                    nc.tensor.matmul(
                        pp[:, 0:w],
                        wqk[:, c, m, t, :],
                        xr[:, c, c0:c1],
                        start=(c == 0), stop=(c == 1),
                    )
                if eng == 'act':
                    nc.scalar.add(dst[:, m, c0:c1], pp[:, 0:w], bt[:, t, m:m + 1])
                else:
                    nc.vector.tensor_scalar_add(
                        dst[:, m, c0:c1], pp[:, 0:w], bt[:, t, m:m + 1]
                    )

            # pts is PACKED: chunk i's live columns [128i, S) stored at
            # [POFF[i], POFF[i+1]), so merged exps write contiguous ranges
            POFF = [0, 1024, 1920, 2688, 3328, 3840, 4224, 4480, 4608]
            pts_tiles = {}

            def get_pts(h):
                if h not in pts_tiles:
                    pts = ptp.tile([128, POFF[8]], BF16, tag="pts",
                                   name=f"pts{h}")
                    pts_tiles[h] = pts
                return pts_tiles[h]

            def pcol(i, c):
                # pts column for chunk i, sq position c
                return POFF[i] + c - 128 * i

            def sc_win(h, i, ps, off, j):
                # score matmuls for sq window j of sk chunk i (+ PE mask on
                # the diagonal block window)
                m, r = h // 4, h % 4
                rows = slice(32 * r, 32 * r + 32)
                we = 512 * (j + 1)
                if we <= 128 * i:
                    return
                ws = max(512 * j, 128 * i)
                ws_mm = max(min(ws, we - 256), 512 * j)
                diag = ws == 128 * i  # window containing the diagonal block
                nc.tensor.matmul(
                    ps[:, ws_mm - off:we - off],
                    k_sb[rows, m, 128 * i:128 * (i + 1)],
                    q_sb[rows, m, ws_mm:we],
                    start=True, stop=not diag,
                    tile_position=(32 * r, 0),
                )
                if diag:
                    # psum[sk, sq] += -1e30 for sq < sk within the block
                    nc.tensor.matmul(
                        ps[:, 128 * i - off:128 * (i + 1) - off],
                        ident, mlow,
                        start=False, stop=True,
                    )

            def sc_exp(h, i, ps, off, e0, e1):
                nc.scalar.activation(
                    get_pts(h)[:, pcol(i, e0):pcol(i, e1)],
                    ps[:, e0 - off:e1 - off], EXP
                )

            def sc(h, i):
                # steady-state chunk 0-3: both windows + one exp (big ring)
                get_pts(h)
                ps = big.tile([128, S], F32, tag="bg", name=f"ps{h}_{i}")
                for j in range(2):
                    sc_win(h, i, ps, 0, j)
                sc_exp(h, i, ps, 0, 128 * i, S)

            def sc45(h):
                # chunks 4+5 share a big tile (c4 at [0:512], c5 at
                # [512:896]) and ONE exp over the packed pts range
                m, r = h // 4, h % 4
                rows = slice(32 * r, 32 * r + 32)
                ps = big.tile([128, S], F32, tag="bg", name=f"ps{h}_45")
                nc.tensor.matmul(ps[:, 0:512], k_sb[rows, m, 512:640],
                                 q_sb[rows, m, 512:1024], start=True,
                                 stop=False, tile_position=(32 * r, 0))
                nc.tensor.matmul(ps[:, 0:128], ident, mlow,
                                 start=False, stop=True)
                nc.tensor.matmul(ps[:, 512:896], k_sb[rows, m, 640:768],
                                 q_sb[rows, m, 640:1024], start=True,
                                 stop=False, tile_position=(32 * r, 0))
                nc.tensor.matmul(ps[:, 512:640], ident, mlow,
                                 start=False, stop=True)
                nc.scalar.activation(
                    get_pts(h)[:, POFF[4]:POFF[6]], ps[:, 0:896], EXP)

            def sc67(h):
                # chunks 6+7 share a small tile (c6 at [0:256], c7 at
                # [256:384], unwidened) and ONE exp
                m, r = h // 4, h % 4
                rows = slice(32 * r, 32 * r + 32)
                ps = sml.tile([128, 512], F32, tag="c67", name=f"ps{h}_67")
                nc.tensor.matmul(ps[:, 0:256], k_sb[rows, m, 768:896],
                                 q_sb[rows, m, 768:1024], start=True,
                                 stop=False, tile_position=(32 * r, 0))
                nc.tensor.matmul(ps[:, 0:128], ident, mlow,
                                 start=False, stop=True)
                nc.tensor.matmul(ps[:, 256:384], k_sb[rows, m, 896:1024],
                                 q_sb[rows, m, 896:1024], start=True,
                                 stop=False, tile_position=(32 * r, 0))
                nc.tensor.matmul(ps[:, 256:384], ident, mlow,
                                 start=False, stop=True)
                nc.scalar.activation(
                    get_pts(h)[:, POFF[6]:POFF[8]], ps[:, 0:384], EXP)

            def att_mms(h, pa, ii, q0, q1, first, last, base=0):
                # accumulate sq columns [q0, q1) into pa[:, q0-base:q1-base]
                pts = pts_tiles[h]
                for idx, i in enumerate(ii):
                    ws = max(q0, 128 * i)
                    nc.tensor.matmul(
                        pa[:, ws - base:q1 - base],
                        vaug[:, i, h, :],
                        pts[:, pcol(i, ws):pcol(i, q1)],
                        start=(first and idx == 0),
                        stop=(last and idx == len(ii) - 1),
                    )

            def att_recip_bcast(pa, p0, p1):
                w = p1 - p0
                rf = rbp.tile([1, 512], F32, tag="rf")
                nc.vector.reciprocal(rf[:, 0:w], pa[32:33, p0:p1])
                rb = rbp.tile([32, 512], F32, tag="rb")
                nc.gpsimd.partition_broadcast(rb[:, 0:w], rf[:, 0:w])
                return rb

            def att_mul(h, pa, rb, q0, q1, p0=0):
                # att columns [q0, q1) normalized from pa[0:32, p0:p0+w]
                m, r = h // 4, h % 4
                w = q1 - q0
                nc.vector.tensor_mul(
                    att[32 * r:32 * r + 32, m, q0:q1],
                    pa[0:32, p0:p0 + w], rb[:, 0:w],
                )

            out_ap = outd.ap().rearrange("(m p) s -> p m s", p=128)

            def po_piece(q0, q1, eng):
                # output projection for columns [q0, q1); the bias is folded
                # in as a bias-row x ones accumulate matmul, so the psum ->
                # SBUF move is a single plain copy per piece
                w = q1 - q0
                po = big.tile([128, 2, 512], F32, tag="bg", name=f"po{q0}")
                for m in range(2):
                    for c in range(2):
                        nc.tensor.matmul(
                            po[:, m, 0:w],
                            wp[:, c, m * 128:(m + 1) * 128],
                            att[:, c, q0:q1],
                            start=(c == 0), stop=False,
                        )
                    nc.tensor.matmul(
                        po[:, m, 0:w],
                        bprow[:, m * 128:(m + 1) * 128],
                        ones[:, 0:w],
                        start=False, stop=True,
                    )
                ot = osb.tile([128, 2, 512], F32, tag="ot", name=f"ot{q0}")
                if eng == 'act':
                    nc.scalar.copy(ot[:, :, 0:w], po[:, :, 0:w])
                else:
                    nc.vector.tensor_copy(ot[:, :, 0:w], po[:, :, 0:w])
                nc.sync.dma_start(out=out_ap[:, :, q0:q1], in_=ot[:, :, 0:w])

            # ---------------- head 0 priming ----------------
            def sc_win0(i, ps, off, w0, w1):
                # head-0 score mms + exp for sq window [w0, w1) of chunk i
                ws = max(w0, 128 * i)
                if w1 <= ws:
                    return
                ws_mm = max(min(ws, w1 - 256), w0)
                diag = ws == 128 * i
                nc.tensor.matmul(
                    ps[:, ws_mm - off:w1 - off],
                    k_sb[0:32, 0, 128 * i:128 * (i + 1)],
                    q_sb[0:32, 0, ws_mm:w1],
                    start=True, stop=not diag,
                    tile_position=(0, 0),
                )
                if diag:
                    nc.tensor.matmul(
                        ps[:, 128 * i - off:128 * (i + 1) - off],
                        ident, mlow,
                        start=False, stop=True,
                    )
                nc.scalar.activation(
                    get_pts(0)[:, pcol(i, ws):pcol(i, w1)],
                    ps[:, ws - off:w1 - off], EXP
                )

            qk_proj(0, 0, 0, 256, 'dve')
            qk_proj(1, 0, 0, 256, 'act')
            b0 = big.tile([128, S], F32, tag="bg", name="ps0_0")
            b1 = big.tile([128, S], F32, tag="bg", name="ps0_1")
            sc_win0(0, b0, 0, 0, 256)
            sc_win0(1, b1, 0, 0, 256)
            qk_proj(0, 0, 256, 512, 'dve')
            qk_proj(1, 0, 256, 512, 'act')
            sc_win0(0, b0, 0, 256, 512)
            sc_win0(1, b1, 0, 256, 512)
            # chunks 2/3's j0 windows share one small tile (disjoint cols)
            s23a = sml.tile([128, 512], F32, tag="sm", name="ps0_23a")
            sc_win0(2, s23a, 256, 256, 512)
            sc_win0(3, s23a, 0, 256, 512)
            # j1 windows (x second half); chunks 2/3 share one big tile
            qk_proj(0, 0, 512, 1024, 'dve')
            qk_proj(1, 0, 512, 1024, 'act')
            sc_win0(0, b0, 0, 512, 1024)
            sc_win0(1, b1, 0, 512, 1024)
            s23b = big.tile([128, S], F32, tag="bg", name="ps0_23b")
            sc_win0(2, s23b, 512, 512, 1024)
            sc_win0(3, s23b, 0, 512, 1024)
            sc45(0)
            sc(1, 0)
            sc67(0)

            # v projection (after wv lands) + head-0 attnv j0
            def v_proj(i):
                pv = pap.tile([128, 512], F32, tag="pa", name=f"pv{i}")
                for c in range(2):
                    nc.tensor.matmul(
                        pv[:, 0:256],
                        xr[:, c, i * 128:(i + 1) * 128],
                        wv[:, c, :],
                        start=(c == 0), stop=(c == 1),
                    )
                nc.vector.tensor_copy(
                    vaug[:, i, :, 0:D],
                    pv[:, 0:256].rearrange("p (h d) -> p h d", h=H),
                )

            # m1 q/k projections: wqk1 lands mid-head-0; doing these here
            # keeps their psum slots and bias-adds off the steady-state path
            qk_proj(0, 1, 0, 512, 'dve')
            qk_proj(1, 1, 0, 512, 'dve')
            qk_proj(0, 1, 512, 1024, 'dve')
            qk_proj(1, 1, 512, 1024, 'dve')

            for i in range(8):
                v_proj(i)

            # One [33,512] accumulator per head: j0 round in head h, then the
            # j1 round REUSES the same tile in h+1 (after the j0 multiply)
            pa_att = {}  # h -> accumulator tile
            pa_j0 = {}   # h -> rb for the j0 half

            def attn_j0(h):
                pa = pap.tile([33, 512], F32, tag="pa", name=f"pa{h}")
                pa_att[h] = pa
                att_mms(h, pa, [0, 1, 2, 3], 0, 512, True, True)
                pa_j0[h] = att_recip_bcast(pa, 0, 512)

            attn_j0(0)
            att_mul(0, pa_att[0], pa_j0.pop(0), 0, 512)

            # ---------------- steady heads ----------------
            def emit_steady(h):
                prev = h - 1
                for i in (1, 2, 3):
                    sc(h, i)
                sc45(h)
                # attnv j0 of h: mms ready after e3; PE reaches them here
                attn_j0(h)
                sc(h + 1, 0)
                sc67(h)
                # j0 multiply of h (dep: bcast just emitted)
                att_mul(h, pa_att[h], pa_j0.pop(h), 0, 512)
                # attnv j1 of prev, reusing its accumulator (j0 mul done)
                pa_prev = pa_att.pop(prev)
                att_mms(prev, pa_prev, [0, 1, 2, 3], 512, 1024, True, False,
                        base=512)
                att_mms(prev, pa_prev, [4, 5, 6, 7], 512, 1024, False, True,
                        base=512)
                rbj1 = att_recip_bcast(pa_prev, 0, 512)
                att_mul(prev, pa_prev, rbj1, 512, 1024, p0=0)
                pts_tiles.pop(prev)

            for h in range(1, 7):
                emit_steady(h)

            # ---------------- head 7 + tail ----------------
            p7 = get_pts(7)
            pa6 = pa_att.pop(6)
            sc(7, 1)
            sc(7, 2)
            # j1 of head 6, interleaved with head-7 scores
            att_mms(6, pa6, [0, 1], 512, 1024, True, False, base=512)
            sc(7, 3)
            att_mms(6, pa6, [2, 3], 512, 1024, False, False, base=512)
            sc45(7)
            # attnv j0 of head 7, group A: cols [0:256) needs chunks 0,1
            pa7 = pap.tile([33, 512], F32, tag="pa", name="pa7")
            nc.tensor.matmul(pa7[:, 0:256], vaug[:, 0, 7, :], p7[:, 0:256],
                             start=True, stop=False)
            nc.tensor.matmul(pa7[:, 128:256], vaug[:, 1, 7, :],
                             p7[:, pcol(1, 128):pcol(1, 256)],
                             start=False, stop=True)
            rb_a = att_recip_bcast(pa7, 0, 256)
            att_mms(6, pa6, [4, 5, 6, 7], 512, 1024, False, True, base=512)
            rbj1_6 = att_recip_bcast(pa6, 0, 512)
            # group B: cols [256:512) needs chunks 0-3
            nc.tensor.matmul(pa7[:, 256:512], vaug[:, 0, 7, :], p7[:, 256:512],
                             start=True, stop=False)
            nc.tensor.matmul(pa7[:, 256:512], vaug[:, 1, 7, :],
                             p7[:, pcol(1, 256):pcol(1, 512)],
                             start=False, stop=False)
            nc.tensor.matmul(pa7[:, 256:512], vaug[:, 2, 7, :],
                             p7[:, pcol(2, 256):pcol(2, 512)],
                             start=False, stop=False)
            nc.tensor.matmul(pa7[:, 384:512], vaug[:, 3, 7, :],
                             p7[:, pcol(3, 384):pcol(3, 512)],
                             start=False, stop=True)
            rb_b = att_recip_bcast(pa7, 256, 512)
            # tail accumulators: [512:768) at cols [0:256) and [768:1024)
            # at cols [256:512) of one shared small-ring tile.  Everything
            # that only needs e45 is emitted BEFORE sc67 so its sem waits
            # bind to early exps, and the [6,7] matmuls + recip-de come
            # right after sc67, ahead of the po pieces.
            pa_cde = sml.tile([33, 512], F32, tag="sm", name="pa_cde")
            att_mms(7, pa_cde, [0, 1, 2, 3, 4, 5], 512, 768, True, True,
                    base=512)
            rb_c = att_recip_bcast(pa_cde, 0, 256)
            att_mms(7, pa_cde, [0, 1, 2, 3, 4, 5], 768, 1024, True, False,
                    base=512)
            sc67(7)
            pts_tiles.pop(6)
            att_mms(7, pa_cde, [6, 7], 768, 1024, False, True, base=512)
            rb_de = att_recip_bcast(pa_cde, 256, 512)
            # muls + outproj pieces, pipelined against the exp tail
            att_mul(7, pa7, rb_a, 0, 256)
            po_piece(0, 256, 'act')
            att_mul(6, pa6, rbj1_6, 512, 1024, p0=0)
            att_mul(7, pa7, rb_b, 256, 512, p0=256)
            po_piece(256, 512, 'act')
            att_mul(7, pa_cde, rb_c, 512, 768, p0=0)
            po_piece(512, 768, 'act')
            att_mul(7, pa_cde, rb_de, 768, 1024, p0=256)
            po_piece(768, 1024, 'act')

    nc.compile()
    return nc


def get_program():
    if "nc" not in _CACHE:
        _CACHE["nc"] = _build_program()
    return _CACHE["nc"]


def kernel(x, wq, bq, wkv, bkv, wproj, bproj):
    import ml_dtypes
    from concourse.bass_utils import run_bass_kernel_spmd

    nc = get_program()

    x = np.asarray(x, dtype=np.float32)
    n = x.shape[0]
    assert n == N_CORES and x.shape[1:] == (C, 32, 32)

    scale = 1.0 / np.sqrt(np.float32(D))
    wq_s = np.asarray(wq, np.float32) * scale
    bq_s = np.asarray(bq, np.float32) * scale
    wk = np.asarray(wkv[:E], np.float32)
    bk = np.asarray(bkv[:E], np.float32)
    wv = np.asarray(wkv[E:], np.float32)
    bv = np.asarray(bkv[E:], np.float32)
    wproj = np.asarray(wproj, np.float32)
    bproj_eff = (np.asarray(bproj, np.float32)
                 + wproj.astype(np.float64) @ bv.astype(np.float64)).astype(np.float32)

    # msk2: [identity | -1e30 * strict_lower(r > sq)]
    ident = np.eye(128, dtype=np.float32)
    mlow = np.where(np.arange(128)[:, None] > np.arange(128)[None, :],
                    np.float32(-1e30), np.float32(0.0))
    msk2 = np.concatenate([ident, mlow], axis=1).astype(ml_dtypes.bfloat16)

    shared = {
        "wqk0": np.ascontiguousarray(
            np.concatenate([wq_s.T[:, 0:128], wk.T[:, 0:128]], axis=1)),
        "wqk1": np.ascontiguousarray(
            np.concatenate([wq_s.T[:, 128:256], wk.T[:, 128:256]], axis=1)),
        "wvt": np.ascontiguousarray(wv.T),
        "wpt": np.ascontiguousarray(wproj.T.astype(ml_dtypes.bfloat16)),
        "biasd": np.ascontiguousarray(
            np.concatenate([bq_s, bk, bproj_eff])),
        "bprow": np.ascontiguousarray(
            bproj_eff.reshape(1, 256).astype(ml_dtypes.bfloat16)),
        "msk2": np.ascontiguousarray(msk2),
    }
    in_maps = [
        {"xin": np.ascontiguousarray(x[i].reshape(C, S)), **shared}
        for i in range(N_CORES)
    ]
    res = run_bass_kernel_spmd(nc, in_maps, core_ids=list(range(N_CORES)))
    out = np.stack([res.results[i]["out"].reshape(O, 32, 32) for i in range(N_CORES)])
    return out.astype(np.float32)
